# revision 1
# baseline (speedup 1.0000x reference)
"""Trainium2 Bass kernel for nn_LCN (locally-connected network).

Computation (see module docstring math):
  x: (512, 1, 280, 280) -> non-overlapping 28x28 patches (10x10 grid, P=100)
  y[b, f, p] = sum_{k,l} x[b, 28ph+k, 28pw+l] * w[f*100+p, 0, k, l]
  y = relu(y + bias[f*100+p]);  out = y_flat @ dec_w.T + dec_b   (j = f*100 + p)

Sharding: 8 cores = 4 batch groups x 2 image halves (rows 0..139 | 140..279).
Each core: 128 images, 5 bands (28 rows each), 50 patches.
Per core pipeline:
  - DMA band [128b, 7840] (fp32, contiguous in HBM)
  - PE transpose x chunks [128b, 112pix] -> PSUM [112, 128] (identity matmul)
  - DVE/ACT evacuate PSUM -> SBUF
  - per patch: 7 accumulating matmuls lhsT=w[112,16], rhs=xT[112,128] -> y PSUM
    (4 patches per PSUM tile at partition offsets 0/32/64/96)
  - ACT: relu(y + bias) -> y_sb  [j on partitions, gappy layout]
  - decoder: 13 accumulating matmuls lhsT=dec[128,10] (zeros in gaps) -> out [10,128]
Host sums the two half-image partial decoder outputs and adds dec_b.
"""

import sys

import numpy as np

for _p in ("/opt/trn_rl_repo", "/opt/trn_rl_repo/concourse"):
    if _p not in sys.path:
        sys.path.insert(0, _p)

import concourse.bass as bass
import concourse.mybir as mybir
import concourse.tile as tile
from concourse import bacc
from concourse.masks import make_identity

F32 = mybir.dt.float32

# Problem constants
B, H, W = 512, 280, 280
KS = 28
HS = WS = 10
F = 16
OUT = 10
NCORES = 8
BLOC = 128      # images per core
NBANDS = 5      # bands per core (half image)
NPW = 10        # patches per band
NCHUNK = 7      # 112-pixel chunks per patch (4 rows x 28 cols each)
CK = 112        # contraction chunk size
BAND_W = KS * W  # 7840 elements per band per image


def build_program(n_bands=NBANDS, n_pw=NPW, use_is_transpose=True, use_bf16=False):
    np_loc = n_bands * n_pw
    ng = (np_loc + 3) // 4
    WDT = mybir.dt.bfloat16 if use_bf16 else F32

    nc = bacc.Bacc("TRN2")
    x_d = nc.dram_tensor("x", [BLOC, n_bands * BAND_W], F32, kind="ExternalInput")
    w_d = nc.dram_tensor("w", [CK, np_loc * NCHUNK * F], WDT, kind="ExternalInput")
    b_d = nc.dram_tensor("bias", [128, ng], F32, kind="ExternalInput")
    d_d = nc.dram_tensor("dec", [128, ng * OUT], F32, kind="ExternalInput")
    o_d = nc.dram_tensor("out", [OUT, BLOC], F32, kind="ExternalOutput")

    with tile.TileContext(nc) as tc:
        with (
            tc.tile_pool(name="const", bufs=1) as constp,
            tc.tile_pool(name="xb", bufs=2) as xbp,
            tc.tile_pool(name="xpm", bufs=2) as xpmp,
            tc.tile_pool(name="xt", bufs=3) as xtp,
            tc.tile_pool(name="xtps", bufs=2, space="PSUM") as xtpsp,
            tc.tile_pool(name="yps", bufs=2, space="PSUM") as ypsp,
            tc.tile_pool(name="ops", bufs=1, space="PSUM") as opsp,
        ):
            ident = constp.tile([128, 128], F32)
            make_identity(nc, ident[:])
            zero_sb = constp.tile([128, 128], F32)
            nc.gpsimd.memset(zero_sb[:], 0.0)
            w_sb = constp.tile([CK, np_loc * NCHUNK * F], WDT)
            nc.sync.dma_start(out=w_sb[:], in_=w_d[:])
            bias_sb = constp.tile([128, ng], F32)
            nc.sync.dma_start(out=bias_sb[:], in_=b_d[:])
            dec_sb = constp.tile([128, ng * OUT], F32)
            nc.sync.dma_start(out=dec_sb[:], in_=d_d[:])
            y_sb = constp.tile([128, ng * 128], F32)

            x_tiles = {}
            xpm_tiles = {}

            def load_band(b):
                t = xbp.tile([128, BAND_W], F32, name="x_sb")
                nc.sync.dma_start(out=t[:], in_=x_d[:, b * BAND_W:(b + 1) * BAND_W])
                x_tiles[b] = t

            def im2col(b):
                # reorder band [b, (k pw l)] -> patch-major [b, (pw k l)]
                # so transpose lhsT chunks are contiguous (walrus: 1 free dim)
                t = xpmp.tile([128, BAND_W], F32, name="x_pm")
                src = x_tiles[b][:].rearrange(
                    "b (k pw l) -> b pw k l", k=KS, pw=NPW)
                if b % 2 == 0:
                    nc.vector.tensor_copy(t[:], src)
                else:
                    nc.scalar.activation(
                        out=t[:], in_=src,
                        func=mybir.ActivationFunctionType.Copy)
                xpm_tiles[b] = t
                x_tiles.pop(b)

            y_tiles = {}

            def emit_mms(pp, xtA, xtB):
                G, q = pp // 4, pp % 4
                if G not in y_tiles:
                    yt = ypsp.tile([128, 128], F32, name="y_ps")
                    if G < 2:
                        # clear stale/NaN PSUM so gap partitions are finite
                        nc.vector.tensor_copy(yt[:], zero_sb[:])
                    y_tiles[G] = yt
                yt = y_tiles[G]
                for t in range(NCHUNK):
                    if t < 4:
                        rhs = xtA[:, t * 128:(t + 1) * 128]
                    else:
                        rhs = xtB[:, (t - 4) * 128:(t - 3) * 128]
                    nc.tensor.matmul(
                        yt[32 * q:32 * q + F, :],
                        w_sb[:, (pp * NCHUNK + t) * F:(pp * NCHUNK + t + 1) * F],
                        rhs,
                        start=(t == 0),
                        stop=(t == NCHUNK - 1),
                        tile_position=(0, 32 * q),
                    )
                if q == 3 or pp == np_loc - 1:
                    nc.scalar.activation(
                        out=y_sb[:, G * 128:(G + 1) * 128],
                        in_=yt[:],
                        func=mybir.ActivationFunctionType.Relu,
                        bias=bias_sb[:, G:G + 1],
                    )

            prev = None
            for p in range(np_loc):
                band, pw = p // n_pw, p % n_pw
                if pw == 0:
                    if band == 0:
                        load_band(0)
                        if n_bands > 1:
                            load_band(1)
                        im2col(0)
                    if band + 1 < n_bands:
                        if band + 2 < n_bands:
                            load_band(band + 2)
                        im2col(band + 1)
                x_pm = xpm_tiles[band]
                xtA_ps = xtpsp.tile([CK, 512], F32, name="xtA_ps")
                xtB_ps = xtpsp.tile([CK, 384], F32, name="xtB_ps")
                for t in range(NCHUNK):
                    if t < 4:
                        dst = xtA_ps[:, t * 128:(t + 1) * 128]
                    else:
                        dst = xtB_ps[:, (t - 4) * 128:(t - 3) * 128]
                    src = x_pm[:, pw * 784 + t * CK: pw * 784 + (t + 1) * CK]
                    if use_is_transpose:
                        nc.tensor.transpose(dst, src, ident[:])
                    else:
                        nc.tensor.matmul(dst, src, ident[:])
                xtA = xtp.tile([CK, 512], WDT, name="xtA")
                xtB = xtp.tile([CK, 384], WDT, name="xtB")
                if p % 2 == 0:
                    nc.vector.tensor_copy(xtA[:], xtA_ps[:])
                    nc.vector.tensor_copy(xtB[:], xtB_ps[:])
                else:
                    nc.scalar.activation(
                        out=xtA[:], in_=xtA_ps[:],
                        func=mybir.ActivationFunctionType.Copy)
                    nc.scalar.activation(
                        out=xtB[:], in_=xtB_ps[:],
                        func=mybir.ActivationFunctionType.Copy)
                if prev is not None:
                    emit_mms(*prev)
                prev = (p, xtA, xtB)
            emit_mms(*prev)

            # stage 2: decoder  out[o, b] = sum_j dec[j, o] * y[j, b]
            out_ps = opsp.tile([OUT, BLOC], F32)
            for G in range(ng):
                nc.tensor.matmul(
                    out_ps[:],
                    dec_sb[:, G * OUT:(G + 1) * OUT],
                    y_sb[:, G * 128:(G + 1) * 128],
                    start=(G == 0),
                    stop=(G == ng - 1),
                )
            out_sb = constp.tile([OUT, BLOC], F32)
            nc.vector.tensor_copy(out_sb[:], out_ps[:])
            nc.sync.dma_start(out=o_d[:], in_=out_sb[:])

    return nc


def stage_half(weight, bias, dec_w, h, n_bands=NBANDS, n_pw=NPW):
    """Host-side staging of weights/bias/decoder for image-half h (0 or 1)."""
    np_loc = n_bands * n_pw
    ng = (np_loc + 3) // 4
    weight = np.asarray(weight, np.float32)
    bias = np.asarray(bias, np.float32)
    dec_w = np.asarray(dec_w, np.float32)

    # w: (1600, 1, 28, 28) -> [f, ph, pw, k, l] -> chunks [d=(kk,l), (bl,pw,t,f)]
    w5 = weight.reshape(F, HS, WS, KS, KS)[:, n_bands * h:n_bands * h + n_bands]
    w6 = w5.reshape(F, n_bands, WS, NCHUNK, 4, KS)  # f bl pw t kk l
    wst = np.ascontiguousarray(
        np.transpose(w6, (4, 5, 1, 2, 3, 0))).reshape(CK, np_loc * NCHUNK * F)

    b5 = bias.reshape(F, HS, WS)[:, n_bands * h:n_bands * h + n_bands, :]
    b5 = b5.reshape(F, np_loc)
    bst = np.zeros((128, ng), np.float32)
    d5 = dec_w.reshape(OUT, F, HS, WS)[:, :, n_bands * h:n_bands * h + n_bands, :]
    d5 = d5.reshape(OUT, F, np_loc)
    dst_ = np.zeros((128, ng * OUT), np.float32)
    for pl in range(np_loc):
        G, q = pl // 4, pl % 4
        bst[32 * q:32 * q + F, G] = b5[:, pl]
        dst_[32 * q:32 * q + F, G * OUT:(G + 1) * OUT] = d5[:, :, pl].T
    return wst, bst, dst_


_cache = {}
USE_BF16 = False
USE_IS_TRANSPOSE = True


def _get_nc():
    key = ("nc", USE_BF16, USE_IS_TRANSPOSE)
    if key not in _cache:
        nc = build_program(use_is_transpose=USE_IS_TRANSPOSE, use_bf16=USE_BF16)
        nc.finalize()
        _cache[key] = nc
    return _cache[key]


def make_in_maps(x, weight, bias, dec_w):
    x = np.asarray(x, np.float32)
    stages = [stage_half(weight, bias, dec_w, h) for h in (0, 1)]
    in_maps = []
    for core in range(NCORES):
        bg, h = core // 2, core % 2
        xs = np.ascontiguousarray(
            x[bg * BLOC:(bg + 1) * BLOC, 0, 140 * h:140 * h + 140, :]
        ).reshape(BLOC, NBANDS * BAND_W)
        wst, bst, dst_ = stages[h]
        if USE_BF16:
            import ml_dtypes
            wst = wst.astype(ml_dtypes.bfloat16)
        in_maps.append({"x": xs, "w": wst, "bias": bst, "dec": dst_})
    return in_maps


def combine(results, dec_b):
    out = np.zeros((B, OUT), np.float32)
    for bg in range(4):
        part = results[2 * bg]["out"] + results[2 * bg + 1]["out"]  # (10, 128)
        out[bg * BLOC:(bg + 1) * BLOC] = part.T + np.asarray(dec_b, np.float32)
    return out


def _install_ntff_hook():
    """Provide the missing antenv.axon_hooks module so trace=True works
    under axon (replicates trn_boot._ntff_profile_via_ctypes)."""
    import contextlib
    import ctypes
    import types

    if "antenv.axon_hooks" in sys.modules:
        return
    so_path = "/opt/axon/libaxon_pjrt.so"
    holder = {}
    mod = types.ModuleType("antenv.axon_hooks")
    mod.set_axon_ntff_profile_hook = lambda h: holder.__setitem__("h", h)
    mod.get_axon_ntff_profile_hook = lambda: holder.get("h")
    sys.modules["antenv.axon_hooks"] = mod
    try:
        import antenv
        antenv.axon_hooks = mod
    except ImportError:
        pass

    lib = ctypes.CDLL(so_path)
    if not hasattr(lib, "axon_start_nrt_profile"):
        return
    lib.axon_start_nrt_profile.argtypes = [
        ctypes.POINTER(ctypes.c_int64), ctypes.c_size_t]
    lib.axon_start_nrt_profile.restype = ctypes.c_int64
    lib.axon_stop_nrt_profile.argtypes = [ctypes.c_char_p]
    lib.axon_stop_nrt_profile.restype = ctypes.c_int64

    @contextlib.contextmanager
    def _hook(output_dir, device_ids):
        import jax
        jax.devices()
        if device_ids:
            ids = (ctypes.c_int64 * len(device_ids))(*device_ids)
            rc = lib.axon_start_nrt_profile(ids, len(device_ids))
        else:
            rc = lib.axon_start_nrt_profile(None, 0)
        if rc != 0:
            raise RuntimeError(f"axon_start_nrt_profile rc={rc}")
        try:
            yield
        finally:
            n = lib.axon_stop_nrt_profile(str(output_dir).encode())
            print(f"profile: {n} file(s) written to {output_dir}")

    mod.set_axon_ntff_profile_hook(_hook)


def run(x, weight, bias, dec_w, dec_b, trace=False):
    from concourse import bass_utils
    from concourse.bass_utils import run_bass_kernel_spmd

    if trace:
        _install_ntff_hook()
        # artifact upload needs a bucket that doesn't exist here
        bass_utils.upload_artifacts = lambda tmpdir: tmpdir

    nc = _get_nc()
    in_maps = make_in_maps(x, weight, bias, dec_w)
    r = run_bass_kernel_spmd(nc, in_maps, list(range(NCORES)), trace=trace)
    return combine(r.results, dec_b), r


def kernel(x, weight, bias, dec_w, dec_b):
    out, _ = run(x, weight, bias, dec_w, dec_b, trace=False)
    return out



# revision 6
# speedup vs baseline: 2.6189x; 2.6189x over previous
"""Trainium2 Bass kernel for nn_LCN (locally-connected network) — v2.

Computation:
  x: (512, 1, 280, 280) -> non-overlapping 28x28 patches (10x10 grid, P=100)
  y[b, f, p] = sum_{k,l} x[b, 28ph+k, 28pw+l] * w[f*100+p, 0, k, l]
  y = relu(y + bias[f*100+p]);  out = y_flat @ dec_w.T + dec_b  (j = f*100+p)

Sharding: 8 cores = 2 batch halves (256 imgs) x 4 patch quarters (25 patches).
x is pre-transposed and cast to bf16 on the host into [pt=175, dd=112, b=256]
blocks (pt = patch*7 + chunk, dd = pixel-in-chunk), so the device does ZERO
transposes and half the DMA bytes vs fp32.

Per core pipeline (all matmuls bf16, 1 cycle/row vs fp32's 4):
  - 7 group DMAs (4 patches each; last group 1 patch), ~1.6 MB apiece,
    rearranged in-flight to SBUF [112, (pt b)]
  - per group: 28 accumulating matmuls lhsT=w[112,16], rhs=x[112,256], four
    patches packed per PSUM bank at partition offsets 0/32/64/96
  - ACT: relu(y + bias) -> y_sb bf16
  - decoder matmul accumulates dec[128,10]^T @ y[128,256] -> out_ps [10,256]
Host combines: out[b] = sum over 4 patch-quarters of partial[o,b] + dec_b.
"""

import sys

import numpy as np

for _p in ("/opt/trn_rl_repo", "/opt/trn_rl_repo/concourse"):
    if _p not in sys.path:
        sys.path.insert(0, _p)

import concourse.bass as bass
import concourse.mybir as mybir
import concourse.tile as tile
from concourse import bacc

F32 = mybir.dt.float32
BF16 = mybir.dt.bfloat16

# Problem constants
B, H, W = 512, 280, 280
KS = 28
HS = WS = 10
P = 100
F = 16
OUT = 10
NCORES = 8
NB = 256        # images per core (batch half)
NP = 25         # patches per core (patch quarter)
NT = 7          # 112-pixel chunks per patch
CK = 112        # contraction chunk (4 rows x 28 cols)
NG = 7          # 4-patch groups per core (6 full + 1 single)
XROWS = NP * NT             # 175 pt-blocks
XBLK = CK * NB              # 28672 elems per pt-block


def build_program():
    nc = bacc.Bacc("TRN2")
    x_d = nc.dram_tensor("x", [XROWS, XBLK], BF16, kind="ExternalInput")
    w_d = nc.dram_tensor("w", [CK, NP * NT * F], BF16, kind="ExternalInput")
    b_d = nc.dram_tensor("bias", [128, NG], F32, kind="ExternalInput")
    d_d = nc.dram_tensor("dec", [128, NG * OUT], BF16, kind="ExternalInput")
    o_d = nc.dram_tensor("out", [OUT, NB], F32, kind="ExternalOutput")

    with tile.TileContext(nc) as tc:
        with (
            tc.tile_pool(name="const", bufs=1) as constp,
            tc.tile_pool(name="xg", bufs=7) as xgp,
            tc.tile_pool(name="yps", bufs=4, space="PSUM") as ypsp,
            tc.tile_pool(name="ops", bufs=1, space="PSUM") as opsp,
        ):
            zero_sb = constp.tile([128, NB], F32)
            nc.gpsimd.memset(zero_sb[:], 0.0)
            w_sb = constp.tile([CK, NP * NT * F], BF16)
            nc.gpsimd.dma_start(out=w_sb[:], in_=w_d[:])
            bias_sb = constp.tile([128, NG], F32)
            nc.gpsimd.dma_start(out=bias_sb[:], in_=b_d[:])
            dec_sb = constp.tile([128, NG * OUT], BF16)
            nc.gpsimd.dma_start(out=dec_sb[:], in_=d_d[:])
            y_sb = constp.tile([128, NG * NB], BF16)

            # issue all x group DMAs up front; Tile pipelines compute per group
            xg_tiles = []
            for g in range(NG):
                npg = 4 if g < NG - 1 else NP - 4 * (NG - 1)
                t_ = xgp.tile([CK, 28 * NB], BF16, name="xg")
                src = x_d[g * 28:g * 28 + npg * NT, :].rearrange(
                    "pt (dd b) -> dd pt b", dd=CK)
                dst = t_[:, :npg * NT * NB].rearrange(
                    "dd (pt b) -> dd pt b", pt=npg * NT)
                nc.sync.dma_start(out=dst, in_=src)
                xg_tiles.append(t_)

            out_ps = opsp.tile([128, 512], F32)
            for g in range(NG):
                npg = 4 if g < NG - 1 else NP - 4 * (NG - 1)
                yt = ypsp.tile([128, 512], F32, name="y_ps")
                # make gap partitions (32q+16..32q+32) finite for the ACT read
                nc.vector.tensor_copy(yt[:, :NB], zero_sb[:])
                xg = xg_tiles[g]
                for q in range(npg):
                    pl = 4 * g + q
                    for t in range(NT):
                        nc.tensor.matmul(
                            yt[32 * q:32 * q + F, :NB],
                            w_sb[:, (pl * NT + t) * F:(pl * NT + t + 1) * F],
                            xg[:, (q * NT + t) * NB:(q * NT + t + 1) * NB],
                            start=(t == 0),
                            stop=(t == NT - 1),
                            tile_position=(0, 32 * q),
                        )
                nc.scalar.activation(
                    out=y_sb[:, g * NB:(g + 1) * NB],
                    in_=yt[:, :NB],
                    func=mybir.ActivationFunctionType.Relu,
                    bias=bias_sb[:, g:g + 1],
                )
                nc.tensor.matmul(
                    out_ps[:OUT, :NB],
                    dec_sb[:, g * OUT:(g + 1) * OUT],
                    y_sb[:, g * NB:(g + 1) * NB],
                    start=(g == 0),
                    stop=(g == NG - 1),
                )

            out_sb = constp.tile([OUT, NB], F32)
            nc.vector.tensor_copy(out_sb[:], out_ps[:OUT, :NB])
            nc.sync.dma_start(out=o_d[:], in_=out_sb[:])

    return nc


def stage_quarter(weight, bias, dec_w, qc):
    """Stage weights/bias/decoder for patch quarter qc (patches 25qc..25qc+24)."""
    import ml_dtypes

    weight = np.asarray(weight, np.float32)
    bias = np.asarray(bias, np.float32)
    dec_w = np.asarray(dec_w, np.float32)

    # w: (1600,1,28,28) -> (f, ph, pw, t, k4, l) -> [dd=(k4 l), (p t f)]
    w6 = weight.reshape(F, HS, WS, NT, 4, KS).transpose(4, 5, 1, 2, 3, 0)
    wt = np.ascontiguousarray(
        w6.reshape(CK, P, NT, F)[:, 25 * qc:25 * qc + NP])
    wst = wt.reshape(CK, NP * NT * F).astype(ml_dtypes.bfloat16)

    b5 = bias.reshape(F, P)
    d5 = dec_w.reshape(OUT, F, P)
    bst = np.zeros((128, NG), np.float32)
    dst = np.zeros((128, NG * OUT), np.float32)
    for pl in range(NP):
        g, q = divmod(pl, 4)
        p = 25 * qc + pl
        bst[32 * q:32 * q + F, g] = b5[:, p]
        dst[32 * q:32 * q + F, g * OUT:(g + 1) * OUT] = d5[:, :, p].T
    return wst, bst, dst.astype(ml_dtypes.bfloat16)


def stage_x(x):
    """x (512,1,280,280) f32 -> global (p, t, dd, b) bf16, host-side."""
    import ml_dtypes

    xr = np.asarray(x, np.float32).reshape(B, HS, NT, 4, WS, KS)
    # (b, ph, t, k4, pw, l) -> (ph, pw, t, k4, l, b)
    xt = np.ascontiguousarray(xr.transpose(1, 4, 2, 3, 5, 0))
    return xt.reshape(P, NT, CK, B).astype(ml_dtypes.bfloat16)


_cache = {}


def _get_nc():
    if "nc" not in _cache:
        nc = build_program()
        nc.finalize()
        _cache["nc"] = nc
    return _cache["nc"]


def make_in_maps(x, weight, bias, dec_w):
    xh = stage_x(x)  # (100, 7, 112, 512) bf16
    quarters = [stage_quarter(weight, bias, dec_w, qc) for qc in range(4)]
    in_maps = []
    for core in range(NCORES):
        h, qc = divmod(core, 4)
        xs = np.ascontiguousarray(
            xh[25 * qc:25 * qc + NP, :, :, NB * h:NB * h + NB]
        ).reshape(XROWS, XBLK)
        wst, bst, dst = quarters[qc]
        in_maps.append({"x": xs, "w": wst, "bias": bst, "dec": dst})
    return in_maps


def combine(results, dec_b):
    out = np.zeros((B, OUT), np.float32)
    dec_b = np.asarray(dec_b, np.float32)
    for h in range(2):
        acc = np.zeros((OUT, NB), np.float32)
        for qc in range(4):
            acc += results[h * 4 + qc]["out"]
        out[NB * h:NB * h + NB] = acc.T + dec_b
    return out


def _install_ntff_hook():
    """Provide the missing antenv.axon_hooks module so trace=True works
    under axon (replicates trn_boot._ntff_profile_via_ctypes)."""
    import contextlib
    import ctypes
    import types

    if "antenv.axon_hooks" in sys.modules:
        return
    so_path = "/opt/axon/libaxon_pjrt.so"
    holder = {}
    mod = types.ModuleType("antenv.axon_hooks")
    mod.set_axon_ntff_profile_hook = lambda h: holder.__setitem__("h", h)
    mod.get_axon_ntff_profile_hook = lambda: holder.get("h")
    sys.modules["antenv.axon_hooks"] = mod
    try:
        import antenv
        antenv.axon_hooks = mod
    except ImportError:
        pass

    lib = ctypes.CDLL(so_path)
    if not hasattr(lib, "axon_start_nrt_profile"):
        return
    lib.axon_start_nrt_profile.argtypes = [
        ctypes.POINTER(ctypes.c_int64), ctypes.c_size_t]
    lib.axon_start_nrt_profile.restype = ctypes.c_int64
    lib.axon_stop_nrt_profile.argtypes = [ctypes.c_char_p]
    lib.axon_stop_nrt_profile.restype = ctypes.c_int64

    @contextlib.contextmanager
    def _hook(output_dir, device_ids):
        import jax
        jax.devices()
        if device_ids:
            ids = (ctypes.c_int64 * len(device_ids))(*device_ids)
            rc = lib.axon_start_nrt_profile(ids, len(device_ids))
        else:
            rc = lib.axon_start_nrt_profile(None, 0)
        if rc != 0:
            raise RuntimeError(f"axon_start_nrt_profile rc={rc}")
        try:
            yield
        finally:
            n = lib.axon_stop_nrt_profile(str(output_dir).encode())
            print(f"profile: {n} file(s) written to {output_dir}")

    mod.set_axon_ntff_profile_hook(_hook)


def run(x, weight, bias, dec_w, dec_b, trace=False):
    from concourse import bass_utils
    from concourse.bass_utils import run_bass_kernel_spmd

    if trace:
        _install_ntff_hook()
        # artifact upload needs a bucket that doesn't exist here
        bass_utils.upload_artifacts = lambda tmpdir: tmpdir

    nc = _get_nc()
    in_maps = make_in_maps(x, weight, bias, dec_w)
    r = run_bass_kernel_spmd(nc, in_maps, list(range(NCORES)), trace=trace)
    return combine(r.results, dec_b), r


def kernel(x, weight, bias, dec_w, dec_b):
    out, _ = run(x, weight, bias, dec_w, dec_b, trace=False)
    return out


# revision 11
# speedup vs baseline: 2.7696x; 1.0576x over previous
"""Trainium2 Bass kernel for nn_LCN (locally-connected network) — v2.

Computation:
  x: (512, 1, 280, 280) -> non-overlapping 28x28 patches (10x10 grid, P=100)
  y[b, f, p] = sum_{k,l} x[b, 28ph+k, 28pw+l] * w[f*100+p, 0, k, l]
  y = relu(y + bias[f*100+p]);  out = y_flat @ dec_w.T + dec_b  (j = f*100+p)

Sharding: 8 cores = 2 batch halves (256 imgs) x 4 patch quarters (25 patches).
x is pre-transposed and cast to bf16 on the host into [pt=175, dd=112, b=256]
blocks (pt = patch*7 + chunk, dd = pixel-in-chunk), so the device does ZERO
transposes and half the DMA bytes vs fp32.

Per core pipeline (all matmuls bf16, 1 cycle/row vs fp32's 4):
  - 7 group DMAs (4 patches each; last group 1 patch), ~1.6 MB apiece,
    rearranged in-flight to SBUF [112, (pt b)]
  - per group: 28 accumulating matmuls lhsT=w[112,16], rhs=x[112,256], four
    patches packed per PSUM bank at partition offsets 0/32/64/96
  - ACT: relu(y + bias) -> y_sb bf16
  - decoder matmul accumulates dec[128,10]^T @ y[128,256] -> out_ps [10,256]
Host combines: out[b] = sum over 4 patch-quarters of partial[o,b] + dec_b.
"""

import sys

import numpy as np

for _p in ("/opt/trn_rl_repo", "/opt/trn_rl_repo/concourse"):
    if _p not in sys.path:
        sys.path.insert(0, _p)

import concourse.bass as bass
import concourse.mybir as mybir
import concourse.tile as tile
from concourse import bacc

F32 = mybir.dt.float32
BF16 = mybir.dt.bfloat16

# Problem constants
B, H, W = 512, 280, 280
KS = 28
HS = WS = 10
P = 100
F = 16
OUT = 10
NCORES = 8
NB = 256        # images per core (batch half)
NP = 25         # patches per core (patch quarter)
NT = 7          # 112-pixel chunks per patch
CK = 112        # contraction chunk (4 rows x 28 cols)
NG = 7          # 4-patch groups per core (6 full + 1 single)
XCOLS = NP * NT * NB        # 44800 columns: (p, t, b), b innermost


def build_program():
    nc = bacc.Bacc("TRN2")
    x_d = nc.dram_tensor("x", [CK, XCOLS], BF16, kind="ExternalInput")
    w_d = nc.dram_tensor("w", [CK, NP * NT * F], BF16, kind="ExternalInput")
    b_d = nc.dram_tensor("bias", [128, NG], F32, kind="ExternalInput")
    d_d = nc.dram_tensor("dec", [128, NG * OUT], BF16, kind="ExternalInput")
    o_d = nc.dram_tensor("out", [OUT, NB], F32, kind="ExternalOutput")

    with tile.TileContext(nc) as tc:
        with (
            tc.tile_pool(name="const", bufs=1) as constp,
            tc.tile_pool(name="xg", bufs=7) as xgp,
            tc.tile_pool(name="yps", bufs=4, space="PSUM") as ypsp,
            tc.tile_pool(name="ops", bufs=1, space="PSUM") as opsp,
        ):
            zero_sb = constp.tile([128, NB], F32)
            nc.gpsimd.memset(zero_sb[:], 0.0)
            w_sb = constp.tile([CK, NP * NT * F], BF16)
            nc.gpsimd.dma_start(out=w_sb[:], in_=w_d[:])
            bias_sb = constp.tile([128, NG], F32)
            nc.gpsimd.dma_start(out=bias_sb[:], in_=b_d[:])
            dec_sb = constp.tile([128, NG * OUT], BF16)
            nc.gpsimd.dma_start(out=dec_sb[:], in_=d_d[:])
            y_sb = constp.tile([128, NG * NB], BF16)

            # issue all x group DMAs up front; Tile pipelines compute per group
            xg_tiles = []
            for g in range(NG):
                npg = 4 if g < NG - 1 else NP - 4 * (NG - 1)
                t_ = xgp.tile([CK, 28 * NB], BF16, name="xg")
                c0 = g * 28 * NB
                nc.sync.dma_start(
                    out=t_[:, :npg * NT * NB],
                    in_=x_d[:, c0:c0 + npg * NT * NB])
                xg_tiles.append(t_)

            out_ps = opsp.tile([128, 512], F32)
            for g in range(NG):
                npg = 4 if g < NG - 1 else NP - 4 * (NG - 1)
                yt = ypsp.tile([128, 512], F32, name="y_ps")
                # make gap partitions (32q+16..32q+32) finite for the ACT read
                nc.vector.tensor_copy(yt[:, :NB], zero_sb[:])
                xg = xg_tiles[g]
                for q in range(npg):
                    pl = 4 * g + q
                    for t in range(NT):
                        nc.tensor.matmul(
                            yt[32 * q:32 * q + F, :NB],
                            w_sb[:, (pl * NT + t) * F:(pl * NT + t + 1) * F],
                            xg[:, (q * NT + t) * NB:(q * NT + t + 1) * NB],
                            start=(t == 0),
                            stop=(t == NT - 1),
                            tile_position=(0, 32 * q),
                        )
                nc.scalar.activation(
                    out=y_sb[:, g * NB:(g + 1) * NB],
                    in_=yt[:, :NB],
                    func=mybir.ActivationFunctionType.Relu,
                    bias=bias_sb[:, g:g + 1],
                )
                nc.tensor.matmul(
                    out_ps[:OUT, :NB],
                    dec_sb[:, g * OUT:(g + 1) * OUT],
                    y_sb[:, g * NB:(g + 1) * NB],
                    start=(g == 0),
                    stop=(g == NG - 1),
                )

            out_sb = constp.tile([OUT, NB], F32)
            nc.vector.tensor_copy(out_sb[:], out_ps[:OUT, :NB])
            nc.sync.dma_start(out=o_d[:], in_=out_sb[:])

    return nc


def stage_quarter(weight, bias, dec_w, qc):
    """Stage weights/bias/decoder for patch quarter qc (patches 25qc..25qc+24)."""
    import ml_dtypes

    weight = np.asarray(weight, np.float32)
    bias = np.asarray(bias, np.float32)
    dec_w = np.asarray(dec_w, np.float32)

    # w: (1600,1,28,28) -> (f, ph, pw, t, k4, l) -> [dd=(k4 l), (p t f)]
    w6 = weight.reshape(F, HS, WS, NT, 4, KS).transpose(4, 5, 1, 2, 3, 0)
    wt = np.ascontiguousarray(
        w6.reshape(CK, P, NT, F)[:, 25 * qc:25 * qc + NP])
    wst = wt.reshape(CK, NP * NT * F).astype(ml_dtypes.bfloat16)

    b5 = bias.reshape(F, P)
    d5 = dec_w.reshape(OUT, F, P)
    bst = np.zeros((128, NG), np.float32)
    dst = np.zeros((128, NG * OUT), np.float32)
    for pl in range(NP):
        g, q = divmod(pl, 4)
        p = 25 * qc + pl
        bst[32 * q:32 * q + F, g] = b5[:, p]
        dst[32 * q:32 * q + F, g * OUT:(g + 1) * OUT] = d5[:, :, p].T
    return wst, bst, dst.astype(ml_dtypes.bfloat16)


def stage_x(x):
    """x (512,1,280,280) f32 -> global [dd=112, p, t, b] bf16 (u16 view).

    Two cache-friendly steps: permute to (b, p, t, dd) with the 112-byte
    l-runs contiguous, then 700 small L2-resident [512,112]->[112,512]
    block transposes to get dd-major.
    """
    import ml_dtypes

    xr = np.asarray(x, np.float32).reshape(B, HS, NT, 4, WS, KS)
    # (b, ph, t, k4, pw, l) -> (b, ph, pw, t, k4, l)
    y1 = np.ascontiguousarray(xr.transpose(0, 1, 4, 2, 3, 5))
    y1 = y1.reshape(B, P * NT, CK).astype(ml_dtypes.bfloat16).view(np.uint16)
    g = np.empty((CK, P * NT, B), np.uint16)
    for pt in range(P * NT):
        g[:, pt, :] = y1[:, pt, :].T
    return g.reshape(CK, P, NT, B)


_cache = {}


def _get_nc():
    if "nc" not in _cache:
        nc = build_program()
        nc.finalize()
        _cache["nc"] = nc
    return _cache["nc"]


def make_in_maps(x, weight, bias, dec_w):
    import ml_dtypes

    xh = stage_x(x)  # (112, 100, 7, 512) u16 (bf16 bits)
    quarters = [stage_quarter(weight, bias, dec_w, qc) for qc in range(4)]
    in_maps = []
    for core in range(NCORES):
        h, qc = divmod(core, 4)
        xs = np.ascontiguousarray(
            xh[:, 25 * qc:25 * qc + NP, :, NB * h:NB * h + NB]
        ).reshape(CK, XCOLS).view(ml_dtypes.bfloat16)
        wst, bst, dst = quarters[qc]
        in_maps.append({"x": xs, "w": wst, "bias": bst, "dec": dst})
    return in_maps


def combine(results, dec_b):
    out = np.zeros((B, OUT), np.float32)
    dec_b = np.asarray(dec_b, np.float32)
    for h in range(2):
        acc = np.zeros((OUT, NB), np.float32)
        for qc in range(4):
            acc += results[h * 4 + qc]["out"]
        out[NB * h:NB * h + NB] = acc.T + dec_b
    return out


def _install_ntff_hook():
    """Provide the missing antenv.axon_hooks module so trace=True works
    under axon (replicates trn_boot._ntff_profile_via_ctypes)."""
    import contextlib
    import ctypes
    import types

    if "antenv.axon_hooks" in sys.modules:
        return
    so_path = "/opt/axon/libaxon_pjrt.so"
    holder = {}
    mod = types.ModuleType("antenv.axon_hooks")
    mod.set_axon_ntff_profile_hook = lambda h: holder.__setitem__("h", h)
    mod.get_axon_ntff_profile_hook = lambda: holder.get("h")
    sys.modules["antenv.axon_hooks"] = mod
    try:
        import antenv
        antenv.axon_hooks = mod
    except ImportError:
        pass

    lib = ctypes.CDLL(so_path)
    if not hasattr(lib, "axon_start_nrt_profile"):
        return
    lib.axon_start_nrt_profile.argtypes = [
        ctypes.POINTER(ctypes.c_int64), ctypes.c_size_t]
    lib.axon_start_nrt_profile.restype = ctypes.c_int64
    lib.axon_stop_nrt_profile.argtypes = [ctypes.c_char_p]
    lib.axon_stop_nrt_profile.restype = ctypes.c_int64

    @contextlib.contextmanager
    def _hook(output_dir, device_ids):
        import jax
        jax.devices()
        if device_ids:
            ids = (ctypes.c_int64 * len(device_ids))(*device_ids)
            rc = lib.axon_start_nrt_profile(ids, len(device_ids))
        else:
            rc = lib.axon_start_nrt_profile(None, 0)
        if rc != 0:
            raise RuntimeError(f"axon_start_nrt_profile rc={rc}")
        try:
            yield
        finally:
            n = lib.axon_stop_nrt_profile(str(output_dir).encode())
            print(f"profile: {n} file(s) written to {output_dir}")

    mod.set_axon_ntff_profile_hook(_hook)


def run(x, weight, bias, dec_w, dec_b, trace=False):
    from concourse import bass_utils
    from concourse.bass_utils import run_bass_kernel_spmd

    if trace:
        _install_ntff_hook()
        # artifact upload needs a bucket that doesn't exist here
        bass_utils.upload_artifacts = lambda tmpdir: tmpdir

    nc = _get_nc()
    in_maps = make_in_maps(x, weight, bias, dec_w)
    r = run_bass_kernel_spmd(nc, in_maps, list(range(NCORES)), trace=trace)
    return combine(r.results, dec_b), r


def kernel(x, weight, bias, dec_w, dec_b):
    out, _ = run(x, weight, bias, dec_w, dec_b, trace=False)
    return out
